# revision 50
# baseline (speedup 1.0000x reference)
# Trainium2 Bass kernel for: ConvTranspose2d(64->128, k=4, stride=1) -> spatial
# mean -> +biases -> 10*logsumexp over channels.
#
# Math: with full (K-1) output padding, the mean over the ENTIRE conv-transpose
# output spatial extent sees every input pixel through all K*K taps, so
#   pooled[n,co] = (sum_hw x[n,ci,hw]) @ (sum_kk w[ci,co,kk]) / (Ho*Wo) + cb + eb
# exactly. The conv collapses to a spatial sum + a (Cin x Cout) matmul.
#
# Sharding: data-parallel over batch N=32 across 8 cores (4 batches/core),
# params replicated.
#
# Trace-driven design (36us baseline -> ~21.7us):
# - x ships as fp8 e4m3 (element rounding lands on a conv term that is only
#   ~1e-3 of the logsumexp input -> ~2e-5 output rel err vs the 2e-2
#   budget). The weight also ships fp8, pre-scaled by 64 on the host (an
#   exact power-of-2 exponent shift) to sit in e4m3's normal range; 1/64
#   is folded back into the device-side mean scale. Halving/quartering
#   bytes matters double here: the DMA stream runs ~70-150 GB/s for its
#   first ~4us before ramping to ~430 GB/s.
# - x streams on ONE HWDGE ring (sync): the DMA engines drain the sync
#   ring completely before serving the scalar ring, so a 2-ring x split
#   just serializes. Per-chunk dispatches/semaphores let each chunk's
#   reduce start the moment it lands. The weight rides the Pool SWDGE
#   queue instead, keeping the slow first microseconds of the sync ring
#   entirely for x.
# - reduces run at 1 elem/lane/cycle everywhere (no DVE 2x/4x modes for
#   reduce-class ops), so the spatial sums are split: DVE runs fused
#   scalar_tensor_tensor ops (add two half-chunks + free-axis accumulate
#   in one pass, ~0.52 ns/col) and ACT runs Copy-with-accumulator on the
#   chunks DVE can't reach in time. Widths are balanced so both engines
#   finish together just after the last chunk lands.
# - the weight k-sum uses a host-side (128, 1024) layout (both co-halves
#   stacked on partitions) so the grouped reduce runs at half the free
#   size; the PE operand is then assembled by four small on-chip quadrant
#   DMAs that depend ONLY on that reduce (the 1/(Ho*Wo*64) scale is folded
#   into the tiny s2m combines, keeping the matmul's dependency chain
#   short and off the erratic Pool engine).
# - ACT table set "natural_log_exp_and_others" covers Copy+Exp+Ln; one
#   preload at the top, no reloads on the Exp/Ln tail. The x10 runs on ACT
#   right after Ln (no cross-engine hop); the y DMA dispatches from SP.
# - known dead ends (hardware): InstTensorTensorReduce and Pool
#   tensor_scalar-with-accumulator fail the ISA check / die at runtime;
#   Pool's first CAST pays a ~1.6us Q7 library load.

import os

import numpy as np
import ml_dtypes

import concourse.bacc as bacc
import concourse.bass as bass
import concourse.mybir as mybir
import concourse.tile as tile
from concourse.bass_utils import run_bass_kernel_spmd
from concourse.hw_specs import get_activation_tables

N, CIN, COUT, K, H, W = 32, 64, 128, 4, 64, 64
NCORES = 8
NLOC = N // NCORES          # 4 batches per core
HW = H * W                  # 4096
ROWS = NLOC * CIN           # 256 rows (n,ci) per core
RBLK = ROWS // 128          # 2 row blocks of 128 partitions
WSCALE = 64.0               # host pre-scale on w (exact in fp)
SCALE = 1.0 / (float((H + K - 1) * (W + K - 1)) * WSCALE)

F32 = mybir.dt.float32
BF16 = mybir.dt.bfloat16
FP8 = mybir.dt.float8e4
NPFP8 = ml_dtypes.float8_e4m3

_CACHE: dict = {}


def _build_module() -> bacc.Bacc:
    nc = bacc.Bacc("TRN2", target_bir_lowering=False, enable_partition_id=False)

    x_d = nc.dram_tensor("xc", [ROWS, HW], FP8, kind="ExternalInput").ap()
    w_d = nc.dram_tensor("w", [128, CIN * K * K], FP8, kind="ExternalInput").ap()
    bs_d = nc.dram_tensor("bs", [2, COUT], F32, kind="ExternalInput").ap()
    y_d = nc.dram_tensor("y", [NLOC, 1], F32, kind="ExternalOutput").ap()

    ADD = mybir.AluOpType.add
    MUL = mybir.AluOpType.mult

    with tile.TileContext(nc) as tc:
        with (
            tc.tile_pool(name="xpool", bufs=5) as xpool,
            tc.tile_pool(name="small", bufs=1) as small,
            tc.tile_pool(name="psum", bufs=1, space="PSUM") as psum_pool,
        ):
            # one ACT table set covering Copy AND Exp AND Ln, loaded once
            act_tables = get_activation_tables(nc.m.arch)
            set_id = next(
                i
                for i, (_, funcs) in enumerate(act_tables.items())
                if mybir.ActivationFunctionType.Exp in funcs
                and mybir.ActivationFunctionType.Ln in funcs
                and mybir.ActivationFunctionType.Copy in funcs
            )
            nc.scalar.add_instruction(
                mybir.InstLoadActFuncSet(
                    name=nc.get_next_instruction_name(), act_func_set_id=set_id
                )
            )

            lowp = lambda: nc.allow_low_precision(
                reason="conv term is tiny vs bias"
            )

            # ---- everything on the sync HWDGE ring, arrival order ----
            biasrows = small.tile([2, COUT], F32)
            nc.sync.dma_start(out=biasrows, in_=bs_d)
            # weight rides the Pool SWDGE queue (separate SDMA service):
            # keeps the sync ring's slow first ~4us entirely for x
            wk = small.tile([128, CIN * K * K], FP8)
            nc.gpsimd.dma_start(out=wk, in_=w_d)

            def ld(rb, lo, hi):
                xt = xpool.tile([128, hi - lo], FP8, tag="xt")
                nc.sync.dma_start(
                    out=xt, in_=x_d[rb * 128 : (rb + 1) * 128, lo:hi]
                )
                return xt

            a0 = ld(0, 0, 2048)        # DVE pair
            b0 = ld(0, 2048, 4096)     # ACT
            a1 = ld(1, 0, 2048)        # DVE pair
            b1 = ld(1, 2048, 3584)     # DVE pair
            # the small tail chunk rides the SWDGE queue behind the weight:
            # it lands early via the parallel SDMA service and ACT folds it
            # while otherwise idle. (Pushing MORE than this through SWDGE
            # backfires: ~0.4 MiB on q0 starves the sync ring by ~1.5us.)
            c1 = xpool.tile([128, 512], FP8, tag="xt")
            nc.gpsimd.dma_start(out=c1, in_=x_d[128:256, 3584:4096])

            # ---- weight k-sums (DVE) ----
            # w ships as (128, 1024): row p<64 holds (ci=p, co 0:64), row
            # p>=64 holds (ci=p-64, co 64:128) - full 128-lane reduce at half
            # the free size of the naive (64, 2048) layout.
            wsum = small.tile([128, CIN], BF16)
            with lowp():
                nc.vector.reduce_sum(
                    out=wsum,
                    in_=wk.rearrange("p (c k) -> p c k", k=K * K),
                    axis=mybir.AxisListType.X,
                )
            # NOTE: the 1/(Ho*Wo * 64) scale is folded into the s2m combines
            # below, so the quadrant DMAs depend only on the k-sum reduce.
            # assemble the PE operand: wdup[p, co] = ws64[p%64, co] via four
            # small on-chip quadrant DMAs (off critical path)
            wdup = small.tile([128, COUT], BF16)
            for dp in (0, 64):
                nc.sync.dma_start(
                    out=wdup[dp : dp + 64, 0:64], in_=wsum[0:64, :]
                )
                nc.sync.dma_start(
                    out=wdup[dp : dp + 64, 64:128], in_=wsum[64:128, :]
                )

            onesb = small.tile([2, NLOC], F32)
            nc.gpsimd.memset(onesb, 1.0)
            # s2m is the zero-masked (128, 4) lhsT: col n nonzero only on
            # partition half n%2 (s2m[(n%2)*64 + ci, n] = sum_hw x[n,ci,:]).
            s2m = small.tile([128, NLOC], BF16)
            nc.gpsimd.memset(s2m, 0.0)

            # ---- spatial sums across DVE (fused pairs) and ACT ----
            # parts cols: 0=a0, 1=b0 (block0); 2=a1, 3=b1, 4=c1 (block1)
            parts = small.tile([128, 5], F32)
            scr = small.tile([128, 1024], FP8)

            def stt(dst_col, w, in0, in1):
                # fused (in0 + in1) with free-axis accumulator: one DVE pass
                # over two half-chunks (InstTensorScalarPtr encoding; the
                # dedicated InstTensorTensorReduce dies on real hardware)
                nc.vector.scalar_tensor_tensor(
                    out=scr[:, 0:w], in0=in0, scalar=1.0, in1=in1,
                    op0=MUL, op1=ADD,
                    accum_out=parts[:, dst_col : dst_col + 1],
                )

            ascr = small.tile([128, 2048], FP8)
            # c1 arrives first (SWDGE): ACT folds it before b0 shows up
            nc.scalar.activation(
                out=ascr[:, 0:512],
                in_=c1,
                func=mybir.ActivationFunctionType.Copy,
                accum_out=parts[:, 4:5],
            )
            stt(0, 1024, a0[:, 0:1024], a0[:, 1024:2048])
            nc.scalar.activation(
                out=ascr,
                in_=b0,
                func=mybir.ActivationFunctionType.Copy,
                accum_out=parts[:, 1:2],
            )
            stt(2, 1024, a1[:, 0:1024], a1[:, 1024:2048])
            stt(3, 768, b1[:, 0:768], b1[:, 768:1536])

            # combines into the masked lhsT, folding the mean scale:
            # s2m = SCALE * sum(parts cols). Block0's pair runs on ACT
            # (Copy with input scale + accumulator) in parallel with DVE's
            # block1 pair, so the matmul isn't gated on one serial engine.
            cscr = small.tile([128, 3], F32)
            with lowp():
                nc.scalar.activation(
                    out=cscr[0:64, 0:2], in_=parts[0:64, 0:2],
                    func=mybir.ActivationFunctionType.Copy, scale=SCALE,
                    accum_out=s2m[0:64, 0:1],
                )
                nc.scalar.activation(
                    out=cscr[64:128, 0:2], in_=parts[64:128, 0:2],
                    func=mybir.ActivationFunctionType.Copy, scale=SCALE,
                    accum_out=s2m[64:128, 1:2],
                )
                nc.vector.tensor_scalar(
                    out=cscr[0:64, 0:3], in0=parts[0:64, 2:5],
                    scalar1=SCALE, scalar2=0.0, op0=MUL, op1=ADD,
                    accum_out=s2m[0:64, 2:3],
                )
                nc.vector.tensor_scalar(
                    out=cscr[64:128, 0:3], in0=parts[64:128, 2:5],
                    scalar1=SCALE, scalar2=0.0, op0=MUL, op1=ADD,
                    accum_out=s2m[64:128, 3:4],
                )

            # ---- pooled (4, 128) in PSUM: bias matmul + data matmul ----
            pooled = psum_pool.tile([NLOC, COUT], F32, space="PSUM")
            nc.tensor.matmul(
                out=pooled, lhsT=onesb, rhs=biasrows, start=True, stop=False
            )
            nc.tensor.matmul(
                out=pooled, lhsT=s2m, rhs=wdup, start=False, stop=True
            )

            # ---- 10 * log(sum_co exp(pooled)) ----
            expt = small.tile([NLOC, COUT], F32)
            sume = small.tile([NLOC, 1], F32)
            nc.scalar.activation(
                out=expt,
                in_=pooled,
                func=mybir.ActivationFunctionType.Exp,
                accum_out=sume,
            )
            logv = small.tile([NLOC, 1], F32)
            nc.scalar.activation(
                out=logv, in_=sume, func=mybir.ActivationFunctionType.Ln
            )
            # x10 on ACT (no hop after Ln); y dispatch on sync (SP's DMA
            # dispatch is ~2x faster than ACT's)
            outv = small.tile([NLOC, 1], F32)
            nc.scalar.mul(out=outv, in_=logv, mul=10.0)
            nc.sync.dma_start(out=y_d, in_=outv)

    nc.compile()
    return nc


def kernel(x, weight, conv_bias, extra_bias):
    x = np.ascontiguousarray(np.asarray(x, dtype=np.float32))
    weight = np.ascontiguousarray(np.asarray(weight, dtype=np.float32))
    conv_bias = np.ascontiguousarray(np.asarray(conv_bias, dtype=np.float32))
    extra_bias = np.ascontiguousarray(np.asarray(extra_bias, dtype=np.float32))
    assert x.shape == (N, CIN, H, W), x.shape
    assert weight.shape == (CIN, COUT, K, K), weight.shape

    if "nc" not in _CACHE:
        _CACHE["nc"] = _build_module()
    nc = _CACHE["nc"]

    xb = x.reshape(N * CIN, HW).astype(NPFP8)
    wf = weight.reshape(CIN, COUT, K * K) * WSCALE
    w2 = np.ascontiguousarray(
        np.concatenate(
            [
                wf[:, : COUT // 2].reshape(CIN, -1),
                wf[:, COUT // 2 :].reshape(CIN, -1),
            ],
            axis=0,
        ).astype(NPFP8)
    )
    bs2 = np.ascontiguousarray(
        np.stack([conv_bias, extra_bias], axis=0)
    )  # (2, COUT)
    in_maps = []
    for c in range(NCORES):
        xc = np.ascontiguousarray(xb[c * ROWS : (c + 1) * ROWS])
        in_maps.append({"xc": xc, "w": w2, "bs": bs2})

    trace = os.environ.get("BASS_KERNEL_TRACE") == "1"
    res = run_bass_kernel_spmd(
        nc, in_maps, core_ids=list(range(NCORES)), trace=trace
    )
    _CACHE["last_result"] = res
    return np.concatenate([r["y"] for r in res.results], axis=0)


# revision 53
# speedup vs baseline: 1.0339x; 1.0339x over previous
# Trainium2 Bass kernel for: ConvTranspose2d(64->128, k=4, stride=1) -> spatial
# mean -> +biases -> 10*logsumexp over channels.
#
# Math: with full (K-1) output padding, the mean over the ENTIRE conv-transpose
# output spatial extent sees every input pixel through all K*K taps, so
#   pooled[n,co] = (sum_hw x[n,ci,hw]) @ (sum_kk w[ci,co,kk]) / (Ho*Wo) + cb + eb
# exactly. The conv collapses to a spatial sum + a (Cin x Cout) matmul.
#
# Sharding: data-parallel over batch N=32 across 8 cores (4 batches/core),
# params replicated.
#
# Trace-driven design (36us baseline -> ~21.7us):
# - x ships as fp8 e4m3 (element rounding lands on a conv term that is only
#   ~1e-3 of the logsumexp input -> ~2e-5 output rel err vs the 2e-2
#   budget). The weight also ships fp8, pre-scaled by 64 on the host (an
#   exact power-of-2 exponent shift) to sit in e4m3's normal range; 1/64
#   is folded back into the device-side mean scale. Halving/quartering
#   bytes matters double here: the DMA stream runs ~70-150 GB/s for its
#   first ~4us before ramping to ~430 GB/s.
# - x streams on ONE HWDGE ring (sync): the DMA engines drain the sync
#   ring completely before serving the scalar ring, so a 2-ring x split
#   just serializes. Per-chunk dispatches/semaphores let each chunk's
#   reduce start the moment it lands. The weight rides the Pool SWDGE
#   queue instead, keeping the slow first microseconds of the sync ring
#   entirely for x.
# - reduces run at 1 elem/lane/cycle everywhere (no DVE 2x/4x modes for
#   reduce-class ops), so the spatial sums are split: DVE runs fused
#   scalar_tensor_tensor ops (add two half-chunks + free-axis accumulate
#   in one pass, ~0.52 ns/col) and ACT runs Copy-with-accumulator on the
#   chunks DVE can't reach in time. Widths are balanced so both engines
#   finish together just after the last chunk lands.
# - the weight k-sum uses a host-side (128, 1024) layout (both co-halves
#   stacked on partitions) so the grouped reduce runs at half the free
#   size; the PE operand is then assembled by four small on-chip quadrant
#   DMAs that depend ONLY on that reduce (the 1/(Ho*Wo*64) scale is folded
#   into the tiny s2m combines, keeping the matmul's dependency chain
#   short and off the erratic Pool engine).
# - ACT table set "natural_log_exp_and_others" covers Copy+Exp+Ln; one
#   preload at the top, no reloads on the Exp/Ln tail. The x10 runs on ACT
#   right after Ln (no cross-engine hop); the y DMA dispatches from SP.
# - known dead ends (hardware): InstTensorTensorReduce and Pool
#   tensor_scalar-with-accumulator fail the ISA check / die at runtime;
#   Pool's first CAST pays a ~1.6us Q7 library load.

import os

import numpy as np
import ml_dtypes

import concourse.bacc as bacc
import concourse.bass as bass
import concourse.mybir as mybir
import concourse.tile as tile
from concourse.bass_utils import run_bass_kernel_spmd
from concourse.hw_specs import get_activation_tables

N, CIN, COUT, K, H, W = 32, 64, 128, 4, 64, 64
NCORES = 8
NLOC = N // NCORES          # 4 batches per core
HW = H * W                  # 4096
ROWS = NLOC * CIN           # 256 rows (n,ci) per core
RBLK = ROWS // 128          # 2 row blocks of 128 partitions
WSCALE = 64.0               # host pre-scale on w (exact in fp)
SCALE = 1.0 / (float((H + K - 1) * (W + K - 1)) * WSCALE)

F32 = mybir.dt.float32
BF16 = mybir.dt.bfloat16
FP8 = mybir.dt.float8e4
NPFP8 = ml_dtypes.float8_e4m3

_CACHE: dict = {}


def _build_module() -> bacc.Bacc:
    nc = bacc.Bacc("TRN2", target_bir_lowering=False, enable_partition_id=False)

    x_d = nc.dram_tensor("xc", [ROWS, HW], FP8, kind="ExternalInput").ap()
    w_d = nc.dram_tensor("w", [128, CIN * K * K], FP8, kind="ExternalInput").ap()
    bs_d = nc.dram_tensor("bs", [2, COUT], F32, kind="ExternalInput").ap()
    y_d = nc.dram_tensor("y", [NLOC, 1], F32, kind="ExternalOutput").ap()

    ADD = mybir.AluOpType.add
    MUL = mybir.AluOpType.mult

    with tile.TileContext(nc) as tc:
        with (
            tc.tile_pool(name="xpool", bufs=5) as xpool,
            tc.tile_pool(name="small", bufs=1) as small,
            tc.tile_pool(name="psum", bufs=1, space="PSUM") as psum_pool,
        ):
            # one ACT table set covering Copy AND Exp AND Ln, loaded once
            act_tables = get_activation_tables(nc.m.arch)
            set_id = next(
                i
                for i, (_, funcs) in enumerate(act_tables.items())
                if mybir.ActivationFunctionType.Exp in funcs
                and mybir.ActivationFunctionType.Ln in funcs
                and mybir.ActivationFunctionType.Copy in funcs
            )
            nc.scalar.add_instruction(
                mybir.InstLoadActFuncSet(
                    name=nc.get_next_instruction_name(), act_func_set_id=set_id
                )
            )

            lowp = lambda: nc.allow_low_precision(
                reason="conv term is tiny vs bias"
            )

            # ---- everything on the sync HWDGE ring, arrival order ----
            biasrows = small.tile([2, COUT], F32)
            nc.sync.dma_start(out=biasrows, in_=bs_d)
            # weight rides the Pool SWDGE queue (separate SDMA service):
            # keeps the sync ring's slow first ~4us entirely for x
            wk = small.tile([128, CIN * K * K], FP8)
            nc.gpsimd.dma_start(out=wk, in_=w_d)

            def ld(rb, lo, hi):
                xt = xpool.tile([128, hi - lo], FP8, tag="xt")
                nc.sync.dma_start(
                    out=xt, in_=x_d[rb * 128 : (rb + 1) * 128, lo:hi]
                )
                return xt

            a0 = ld(0, 0, 2048)        # DVE pair
            b0 = ld(0, 2048, 4096)     # ACT
            a1 = ld(1, 0, 1792)        # DVE pair
            b1 = ld(1, 1792, 3584)     # DVE pair
            c1 = ld(1, 3584, 4096)     # ACT (small tail)
            # (Routing x chunks through the SWDGE queue was tried: a small
            # tail chunk is neutral, and ~0.4 MiB starves the sync ring by
            # ~1.5us. Only the weight rides SWDGE.)

            # ---- weight k-sums (DVE) ----
            # w ships as (128, 1024): row p<64 holds (ci=p, co 0:64), row
            # p>=64 holds (ci=p-64, co 64:128) - full 128-lane reduce at half
            # the free size of the naive (64, 2048) layout.
            wsum = small.tile([128, CIN], BF16)
            with lowp():
                nc.vector.reduce_sum(
                    out=wsum,
                    in_=wk.rearrange("p (c k) -> p c k", k=K * K),
                    axis=mybir.AxisListType.X,
                )
            # NOTE: the 1/(Ho*Wo * 64) scale is folded into the s2m combines
            # below, so the quadrant DMAs depend only on the k-sum reduce.
            # assemble the PE operand: wdup[p, co] = ws64[p%64, co] via four
            # small on-chip quadrant DMAs (off critical path)
            wdup = small.tile([128, COUT], BF16)
            for dp in (0, 64):
                nc.sync.dma_start(
                    out=wdup[dp : dp + 64, 0:64], in_=wsum[0:64, :]
                )
                nc.sync.dma_start(
                    out=wdup[dp : dp + 64, 64:128], in_=wsum[64:128, :]
                )

            onesb = small.tile([2, NLOC], F32)
            nc.gpsimd.memset(onesb, 1.0)
            # s2m is the zero-masked (128, 4) lhsT: col n nonzero only on
            # partition half n%2 (s2m[(n%2)*64 + ci, n] = sum_hw x[n,ci,:]).
            s2m = small.tile([128, NLOC], BF16)
            nc.gpsimd.memset(s2m, 0.0)

            # ---- spatial sums across DVE (fused pairs) and ACT ----
            # parts cols: 0=a0, 1=b0 (block0); 2=a1, 3=b1, 4=c1 (block1)
            parts = small.tile([128, 5], F32)
            scr = small.tile([128, 1024], FP8)

            def stt(dst_col, w, in0, in1):
                # fused (in0 + in1) with free-axis accumulator: one DVE pass
                # over two half-chunks (InstTensorScalarPtr encoding; the
                # dedicated InstTensorTensorReduce dies on real hardware)
                nc.vector.scalar_tensor_tensor(
                    out=scr[:, 0:w], in0=in0, scalar=1.0, in1=in1,
                    op0=MUL, op1=ADD,
                    accum_out=parts[:, dst_col : dst_col + 1],
                )

            ascr = small.tile([128, 2048], FP8)
            stt(0, 1024, a0[:, 0:1024], a0[:, 1024:2048])
            nc.scalar.activation(
                out=ascr,
                in_=b0,
                func=mybir.ActivationFunctionType.Copy,
                accum_out=parts[:, 1:2],
            )
            stt(2, 896, a1[:, 0:896], a1[:, 896:1792])
            stt(3, 896, b1[:, 0:896], b1[:, 896:1792])
            nc.scalar.activation(
                out=ascr[:, 0:512],
                in_=c1,
                func=mybir.ActivationFunctionType.Copy,
                accum_out=parts[:, 4:5],
            )

            # combines into the masked lhsT (DVE tensor_scalar + accumulator,
            # folding the mean scale): s2m = SCALE * sum(parts cols)
            cscr = small.tile([128, 3], F32)
            with lowp():
                nc.vector.tensor_scalar(
                    out=cscr[0:64, 0:2], in0=parts[0:64, 0:2],
                    scalar1=SCALE, scalar2=0.0, op0=MUL, op1=ADD,
                    accum_out=s2m[0:64, 0:1],
                )
                nc.vector.tensor_scalar(
                    out=cscr[64:128, 0:2], in0=parts[64:128, 0:2],
                    scalar1=SCALE, scalar2=0.0, op0=MUL, op1=ADD,
                    accum_out=s2m[64:128, 1:2],
                )
                nc.vector.tensor_scalar(
                    out=cscr[0:64, 0:3], in0=parts[0:64, 2:5],
                    scalar1=SCALE, scalar2=0.0, op0=MUL, op1=ADD,
                    accum_out=s2m[0:64, 2:3],
                )
                nc.vector.tensor_scalar(
                    out=cscr[64:128, 0:3], in0=parts[64:128, 2:5],
                    scalar1=SCALE, scalar2=0.0, op0=MUL, op1=ADD,
                    accum_out=s2m[64:128, 3:4],
                )

            # ---- pooled (4, 128) in PSUM: bias matmul + data matmul ----
            pooled = psum_pool.tile([NLOC, COUT], F32, space="PSUM")
            nc.tensor.matmul(
                out=pooled, lhsT=onesb, rhs=biasrows, start=True, stop=False
            )
            nc.tensor.matmul(
                out=pooled, lhsT=s2m, rhs=wdup, start=False, stop=True
            )

            # ---- 10 * log(sum_co exp(pooled)) ----
            expt = small.tile([NLOC, COUT], F32)
            sume = small.tile([NLOC, 1], F32)
            nc.scalar.activation(
                out=expt,
                in_=pooled,
                func=mybir.ActivationFunctionType.Exp,
                accum_out=sume,
            )
            logv = small.tile([NLOC, 1], F32)
            nc.scalar.activation(
                out=logv, in_=sume, func=mybir.ActivationFunctionType.Ln
            )
            # x10 on ACT (no hop after Ln); y dispatch on sync (SP's DMA
            # dispatch is ~2x faster than ACT's)
            outv = small.tile([NLOC, 1], F32)
            nc.scalar.mul(out=outv, in_=logv, mul=10.0)
            nc.sync.dma_start(out=y_d, in_=outv)

    nc.compile()
    return nc


def kernel(x, weight, conv_bias, extra_bias):
    x = np.ascontiguousarray(np.asarray(x, dtype=np.float32))
    weight = np.ascontiguousarray(np.asarray(weight, dtype=np.float32))
    conv_bias = np.ascontiguousarray(np.asarray(conv_bias, dtype=np.float32))
    extra_bias = np.ascontiguousarray(np.asarray(extra_bias, dtype=np.float32))
    assert x.shape == (N, CIN, H, W), x.shape
    assert weight.shape == (CIN, COUT, K, K), weight.shape

    if "nc" not in _CACHE:
        _CACHE["nc"] = _build_module()
    nc = _CACHE["nc"]

    xb = x.reshape(N * CIN, HW).astype(NPFP8)
    wf = weight.reshape(CIN, COUT, K * K) * WSCALE
    w2 = np.ascontiguousarray(
        np.concatenate(
            [
                wf[:, : COUT // 2].reshape(CIN, -1),
                wf[:, COUT // 2 :].reshape(CIN, -1),
            ],
            axis=0,
        ).astype(NPFP8)
    )
    bs2 = np.ascontiguousarray(
        np.stack([conv_bias, extra_bias], axis=0)
    )  # (2, COUT)
    in_maps = []
    for c in range(NCORES):
        xc = np.ascontiguousarray(xb[c * ROWS : (c + 1) * ROWS])
        in_maps.append({"xc": xc, "w": w2, "bs": bs2})

    trace = os.environ.get("BASS_KERNEL_TRACE") == "1"
    res = run_bass_kernel_spmd(
        nc, in_maps, core_ids=list(range(NCORES)), trace=trace
    )
    _CACHE["last_result"] = res
    return np.concatenate([r["y"] for r in res.results], axis=0)
